# revision 28
# baseline (speedup 1.0000x reference)
"""Trainium2 Bass kernel for PhysicalConsistencyLoss.

Math summary
------------
Per (batch, channel) the reference builds a 33x33 PSF kernel from three
center-pixel scalars, depthwise-convolves J with it (replicate pad), then

    t      = exp(-beta_D * mu_d)
    B_term = B_inf * (1 - exp(-beta_B * mu_d))
    I_recon = direct * t * L + B_term
    loss    = mean(|I - I_recon|)

The PSF kernel contains a 1/(r^2 + 1e-6) factor, which makes the center tap
carry >= 0.99997 of the normalized mass.  We therefore split

    kernel = c0 * delta + tail          (|tail| <= 2.8e-5 of total)

`c0 * J` is applied exactly in fp32; the tiny tail is applied as a rank-R
separable convolution (SVD of the 33x33 tail, computed on host), evaluated in
bf16 on the tensor engine.  Measured end-to-end error vs. the fp32 reference
is ~6e-6 max-rel on hardware with R=1 (tail singular values decay ~4x/rank;
every extra rank costs DMA weight traffic + PSUM->SBUF copies).

Device mapping (per core = one batch sample)
--------------------------------------------
Separable conv = h-pass then w-pass.  Both passes are regular matmuls with
the *image tile* as the stationary operand (lhsT) and a banded, fold-at-edge
Toeplitz block as the moving operand, so each pass transposes its output:

    pass1:  ZT[w, h'] = sum_h  J[h, w] * Mu[h, h']     (J natural -> ZT)
    pass2: out[h', w'] = sum_w ZT[w, h'] * Mv[w, w']   (ZT -> natural)

Replicate padding is folded into the Toeplitz edge rows, so no data padding
is needed anywhere.  The elementwise epilogue is engine-balanced across
DVE/ACT/GPSIMD (tuned against the v2 instruction cost model), and the loss
uses sum|rec - I| = 2*sum(max(rec, I)) - sum(rec) - sum(I): the two row-sums
come for free as scalar_tensor_tensor accum outputs, and sum(I) is computed
on the host directly from the input (tensor_tensor_reduce would be the
natural op but its custom-DVE ucode crashes this deployment's cores).
Inputs are packed into few large DMAs (J as one 3 MB load; mu/L/scalars in
one aux tensor) and issued in consumption order; I_recon chunks stream out
as soon as each is assembled.

Sharding: pure data parallelism, batch dim across the 8 cores.
"""

import math

import ml_dtypes
import numpy as np

B, C, H, W = 8, 3, 512, 512
KS, PAD = 33, 16
RANK = 2
NCORES = 8
P = 128        # SBUF partitions
NT = H // P    # 4 chunks of 128 rows
BW = PAD * 2 + P  # 160: band width of a Toeplitz chunk block

_bf16 = ml_dtypes.bfloat16


# ----------------------------------------------------------------------------
# Host-side preparation (cheap: 24 x 33x33 kernels + SVDs per call)
# ----------------------------------------------------------------------------

def _psf_kernel(d, b, g):
    coords = np.arange(-PAD, PAD + 1, dtype=np.float64)
    y, x = np.meshgrid(coords, coords, indexing="ij")
    r2 = x * x + y * y + 1e-6
    cos_theta = d / np.sqrt(r2 + d * d + 1e-6)
    g2 = g * g
    phase = (1.0 - g2) / (4.0 * math.pi * (1.0 + g2 - 2.0 * g * cos_theta) ** 1.5)
    k = phase * np.exp(-b * d) / (r2 + 1e-6)
    return k / (k.sum() + 1e-6)


def _fold_M(u):
    """(512, 512) matrix M with out[o] = sum_h M[h, o] * in[h] implementing
    out[o] = sum_dy u[dy] * in[clamp(o + dy - PAD)] (replicate-pad fold)."""
    M = np.zeros((H, H), dtype=np.float64)
    idx = np.arange(H)
    for dy in range(KS):
        rows = np.clip(idx + dy - PAD, 0, H - 1)
        np.add.at(M, (rows, idx), u[dy])
    return M


def _bands(M):
    """Slice M into NT chunk blocks blk[q, p, j] = M[128q + p, 128q - 16 + j]
    (zero where the column index is out of range)."""
    out = np.zeros((NT, P, BW), dtype=np.float64)
    for q in range(NT):
        lo = P * q - PAD
        cols = np.arange(lo, lo + BW)
        v = (cols >= 0) & (cols < H)
        out[q][:, v] = M[P * q:P * q + P, cols[v]]
    return out


def prep_core(I, J, mu, bD, bB, g, L, Binf):
    """Build the per-core input map. I/J/bD: (C,H,W) f32; mu,g,L: (H,W);
    bB, Binf: (C,) f32."""
    hc, wc = H // 2, W // 2
    d = float(mu[hc, wc])
    gg = float(g[hc, wc])
    Wu = np.zeros((C, RANK, 3, P, BW), dtype=np.float64)
    Wv = np.zeros((C, RANK, 3, P, BW), dtype=np.float64)
    scal = np.zeros((P, C, 8), dtype=np.float32)
    for c in range(C):
        k = _psf_kernel(d, float(bD[c, hc, wc]), gg)
        c0 = k[PAD, PAD]
        tail = k.copy()
        tail[PAD, PAD] = 0.0
        U, S, Vt = np.linalg.svd(tail)
        for i in range(RANK):
            s = math.sqrt(S[i])
            Wu[c, i] = _bands(_fold_M(U[:, i] * s))[[0, 1, 3]]
            Wv[c, i] = _bands(_fold_M(Vt[i, :] * s))[[0, 1, 3]]
        scal[:, c, 0] = c0
        scal[:, c, 1] = -float(bB[c])
        scal[:, c, 2] = float(Binf[c])
        scal[:, c, 3] = -float(Binf[c])
    # partition-major for perfectly contiguous DMAs
    Wu = Wu.transpose(3, 0, 1, 2, 4)
    Wv = Wv.transpose(3, 0, 1, 2, 4)
    Wb = np.ascontiguousarray(np.stack([Wu, Wv], axis=1)).astype(_bf16)
    mu_r = np.asarray(mu, np.float32).reshape(NT, P, W).transpose(1, 0, 2)
    L_r = np.asarray(L, np.float32).reshape(NT, P, W).transpose(1, 0, 2)
    aux = np.concatenate(
        [mu_r.reshape(P, -1), L_r.reshape(P, -1), scal.reshape(P, -1)], axis=1)
    return {
        "I": np.ascontiguousarray(I, dtype=np.float32),
        "J": np.ascontiguousarray(J, dtype=np.float32),
        "bD": np.ascontiguousarray(bD, dtype=np.float32),
        "aux": np.ascontiguousarray(aux, dtype=np.float32),
        "Wb": Wb,
    }


# Band-write programs: list of (jlo, jhi, is_first_of_group, is_last_of_group)
# per chunk q, splitting each banded write at the already-written frontier so
# every matmul's destination range is uniformly fresh or uniformly pending
# (PSUM zero-region semantics).
_WSLOT = {0: 0, 1: 1, 2: 1, 3: 2}


def _band_prog(q):
    if q == 0:
        return [(PAD, BW)]                  # cols [0, 144)
    if q in (1, 2):
        return [(0, 2 * PAD), (2 * PAD, BW)]  # cols [128q-16,128q+16)+[.., +144)
    return [(0, 2 * PAD), (2 * PAD, BW - PAD)]  # q == 3: cols up to 512


# ----------------------------------------------------------------------------
# Bass program (single core; SPMD across 8 cores)
# ----------------------------------------------------------------------------

def build_program(variant="full"):
    import concourse.tile as tile
    from concourse import bacc, mybir
    from contextlib import ExitStack

    f32 = mybir.dt.float32
    bf16 = mybir.dt.bfloat16
    AF = mybir.ActivationFunctionType
    OP = mybir.AluOpType

    nc = bacc.Bacc("TRN2", target_bir_lowering=False, debug=False)

    I_d = nc.dram_tensor("I", (C, H, W), f32, kind="ExternalInput")
    J_d = nc.dram_tensor("J", (C, H, W), f32, kind="ExternalInput")
    bD_d = nc.dram_tensor("bD", (C, H, W), f32, kind="ExternalInput")
    aux_d = nc.dram_tensor(
        "aux", (P, 2 * NT * W + C * 8), f32, kind="ExternalInput")
    Wb_d = nc.dram_tensor(
        "Wb", (P, 2, C, RANK, 3, BW), bf16, kind="ExternalInput")
    rec_d = nc.dram_tensor("rec", (C, H, W), f32, kind="ExternalOutput")
    lp_d = nc.dram_tensor("lp", (P, 2, 16), f32, kind="ExternalOutput")

    def body(ctx, tc):
        singles = ctx.enter_context(tc.tile_pool(name="singles", bufs=1))
        chan = ctx.enter_context(tc.tile_pool(name="chan", bufs=2))
        ztp = ctx.enter_context(tc.tile_pool(name="ztp", bufs=2))
        p1p = ctx.enter_context(tc.tile_pool(name="p1p", bufs=2, space="PSUM"))
        p2p = ctx.enter_context(tc.tile_pool(name="p2p", bufs=2, space="PSUM"))
        tmp = ctx.enter_context(tc.tile_pool(name="tmp", bufs=4))

        # All input loads issued up front in consumption order: the DMA ring
        # is the bottleneck, so the last epilogue's inputs must not queue
        # behind anything they don't need, and stores pack the ring tail.
        bD_t, Ii_t = [None] * C, [None] * C
        Jall = chan.tile([P, C, NT, W], f32, name="Jall")
        nc.sync.dma_start(
            out=Jall, in_=J_d.ap().rearrange("c (t p) w -> p c t w", p=P))
        Jn_t = [Jall[:, c, :, :] for c in range(C)]
        Wb_t = singles.tile([P, 2, C, RANK, 3, BW], bf16)
        nc.sync.dma_start(out=Wb_t, in_=Wb_d.ap())
        Wu_t = Wb_t[:, 0]
        Wv_t = Wb_t[:, 1]
        aux_t = singles.tile([P, 2 * NT * W + C * 8], f32)
        nc.sync.dma_start(out=aux_t, in_=aux_d.ap())
        mu_t = aux_t[:, :NT * W].rearrange("p (t w) -> p t w", t=NT)
        L_t = aux_t[:, NT * W:2 * NT * W].rearrange("p (t w) -> p t w", t=NT)
        sc_t = aux_t[:, 2 * NT * W:].rearrange("p (c k) -> p c k", c=C)
        lpA_t = singles.tile([P, 2, 16], f32)
        lpB_t = lpA_t[:, 1, :]
        nc.vector.memset(lpA_t, 0.0)
        for c in range(C):
            bD_t[c] = chan.tile([P, NT, W], f32, tag="bD", bufs=C, name=f"bD{c}")
            Ii_t[c] = chan.tile([P, NT, W], f32, tag="Ii", bufs=C, name=f"Ii{c}")
        for c in range(C):
            nc.sync.dma_start(
                out=bD_t[c], in_=bD_d[c].rearrange("(t p) w -> p t w", p=P))
            nc.sync.dma_start(
                out=Ii_t[c], in_=I_d[c].rearrange("(t p) w -> p t w", p=P))

        for c in range(C):
            Jn, bD, Ii = Jn_t[c], bD_t[c], Ii_t[c]
            Jb = chan.tile([P, NT, W], bf16, tag="Jb")
            nc.gpsimd.tensor_copy(out=Jb, in_=Jn)
            orec = chan.tile([P, NT, W], f32, tag="orec")

            # ---- pass 1: contract h;  ZT_i[w, h'] (per w-chunk) ----
            ZT = [ztp.tile([P, NT, W], bf16, tag=f"zt{i}", name=f"zt{i}")
                  for i in range(RANK)]
            for wc in range(NT if variant == "full" else 0):
                P1 = [p1p.tile([P, W], f32, tag=f"p1_{i}", name=f"p1_{i}")
                      for i in range(RANK)]
                n_seen = [0] * RANK
                n_total = sum(len(_band_prog(q)) for q in range(NT))
                for hcc in range(NT):
                    prog = _band_prog(hcc)
                    base = P * hcc - PAD
                    for i in range(RANK):
                        lhsT = Jb[:, hcc, wc * P:(wc + 1) * P]
                        for (jlo, jhi) in prog:
                            nc.tensor.matmul(
                                P1[i][:, base + jlo:base + jhi],
                                lhsT,
                                Wu_t[:, c, i, _WSLOT[hcc], jlo:jhi],
                                start=(n_seen[i] == 0),
                                stop=(n_seen[i] == n_total - 1),
                            )
                            n_seen[i] += 1
                for i in range(RANK):
                    if i % 2 == 0:
                        nc.vector.tensor_copy(out=ZT[i][:, wc, :], in_=P1[i])
                    else:
                        nc.scalar.copy(out=ZT[i][:, wc, :], in_=P1[i])

            if variant == "dmaonly":
                nc.vector.tensor_copy(out=orec, in_=Jn)
                nc.sync.dma_start(
                    out=rec_d[c].rearrange("(t p) w -> p t w", p=P), in_=orec)
                continue

            # ---- pass 2: contract w; accumulate ranks; epilogue ----
            for t in range(NT):
                P2 = p2p.tile([P, W], f32, tag="p2")
                n = 0
                n_total2 = sum(
                    len(_band_prog(q)) + (RANK - 1) * (1 if q == 0 else 1)
                    for q in range(NT)
                )
                # total instructions: per q: len(prog) for i=0, 1 for each i>0
                n_total2 = sum(len(_band_prog(q)) + (RANK - 1) for q in range(NT))
                for wc in range(NT if variant == "full" else 0):
                    prog = _band_prog(wc)
                    base = P * wc - PAD
                    jlo_full, jhi_full = prog[0][0], prog[-1][1]
                    for i in range(RANK):
                        lhsT = ZT[i][:, wc, t * P:(t + 1) * P]
                        pieces = prog if i == 0 else [(jlo_full, jhi_full)]
                        for (jlo, jhi) in pieces:
                            nc.tensor.matmul(
                                P2[:, base + jlo:base + jhi],
                                lhsT,
                                Wv_t[:, c, i, _WSLOT[wc], jlo:jhi],
                                start=(n == 0),
                                stop=(n == n_total2 - 1),
                            )
                            n += 1

                # epilogue on (128, 512) tile t.  Engine split tuned against
                # the v2 cost model: DVE ~0.6us/op, ACT ~0.5, Pool ~1.16.
                col = c * NT + t
                t1 = tmp.tile([P, W], f32, tag="t1")
                nc.vector.tensor_tensor(
                    out=t1, in0=bD[:, t, :], in1=mu_t[:, t, :], op=OP.mult)
                nc.scalar.activation(out=t1, in_=t1, func=AF.Exp, scale=-1.0)
                nc.gpsimd.tensor_tensor(
                    out=t1, in0=t1, in1=L_t[:, t, :], op=OP.mult)  # t1 = t*L
                t2 = tmp.tile([P, W], f32, tag="t2")
                nc.scalar.activation(
                    out=t2, in_=mu_t[:, t, :], func=AF.Exp, scale=sc_t[:, c, 1:2])
                nc.scalar.activation(
                    out=t2, in_=t2, func=AF.Identity,
                    scale=sc_t[:, c, 3:4], bias=sc_t[:, c, 2:3])  # t2 = B_term
                t3 = tmp.tile([P, W], f32, tag="t3")
                if variant == "full":
                    nc.vector.scalar_tensor_tensor(
                        out=t3, in0=Jn[:, t, :], scalar=sc_t[:, c, 0:1], in1=P2,
                        op0=OP.mult, op1=OP.add)  # t3 = direct
                else:
                    nc.vector.tensor_scalar_mul(t3, Jn[:, t, :], sc_t[:, c, 0:1])
                nc.gpsimd.tensor_tensor(
                    out=t3, in0=t3, in1=t1, op=OP.mult)  # direct * t * L
                # I_recon assembled with a fused row-sum; loss uses
                # sum|rec - I| = 2*sum(max(rec, I)) - sum(rec) - sum(I),
                # with sum(I) computed on the host from the input itself.
                nc.vector.scalar_tensor_tensor(
                    out=orec[:, t, :], in0=t3, scalar=0.0, in1=t2,
                    op0=OP.bypass, op1=OP.add,
                    accum_out=lpA_t[:, 0, col:col + 1])  # I_recon + sum(rec)
                nc.sync.dma_start(
                    out=rec_d[c].rearrange("(t p) w -> p t w", p=P)[:, t, :],
                    in_=orec[:, t, :])
                nc.vector.scalar_tensor_tensor(
                    out=t3, in0=orec[:, t, :], scalar=0.0, in1=Ii[:, t, :],
                    op0=OP.bypass, op1=OP.max,
                    accum_out=lpB_t[:, col:col + 1])  # sum(max(rec, I))

        nc.sync.dma_start(out=lp_d.ap(), in_=lpA_t)

    with tile.TileContext(nc) as tc:
        with ExitStack() as ctx:
            body(ctx, tc)

    nc.compile()
    return nc


_PROG = None


def _get_prog():
    global _PROG
    if _PROG is None:
        _PROG = build_program()
    return _PROG


# ----------------------------------------------------------------------------
# Public entry point
# ----------------------------------------------------------------------------

def make_in_maps(I, J, mu_d, beta_D, beta_B, g, L, B_inf):
    I = np.asarray(I, np.float32)
    J = np.asarray(J, np.float32)
    mu_d = np.asarray(mu_d, np.float32)
    beta_D = np.asarray(beta_D, np.float32)
    beta_B = np.asarray(beta_B, np.float32)
    g = np.asarray(g, np.float32)
    L = np.asarray(L, np.float32)
    B_inf = np.asarray(B_inf, np.float32)
    in_maps = []
    for b in range(B):
        in_maps.append(prep_core(
            I[b], J[b], mu_d[b, 0], beta_D[b],
            beta_B[b, :, 0, 0], g[b, 0], L[b, 0], B_inf[b, :, 0, 0]))
    return in_maps


def assemble_outputs(results, sum_I):
    """sum_I: float64 sum of the full I tensor (computed on host)."""
    rec = np.stack([results[b]["rec"] for b in range(B)], axis=0)
    total = np.float64(0.0)
    for b in range(B):
        lp = np.asarray(results[b]["lp"], np.float64)
        total += 2.0 * lp[:, 1, :C * NT].sum() - lp[:, 0, :C * NT].sum()
    loss = np.float32((total - sum_I) / (B * C * H * W))
    return loss, rec


def kernel(**inputs):
    from concourse.bass_utils import run_bass_kernel_spmd

    nc = _get_prog()
    in_maps = make_in_maps(**inputs)
    res = run_bass_kernel_spmd(nc, in_maps, list(range(NCORES)))
    sum_I = float(np.asarray(inputs["I"], np.float64).sum())
    return assemble_outputs(res.results, sum_I)
